# revision 6
# baseline (speedup 1.0000x reference)
"""Trainium2 Bass kernel for nn_Encoder_17282948399460 (density-based sampler).

Math (per batch):
    density_i = mean_k(8 smallest Euclidean dists from point i)   (self incl.)
    out = features/pos/cam_ids gathered at the top N//5 densities (descending)

Device strategy (pure data parallelism, batch b -> NeuronCore b):
    G[i,j] = 2*f_i.f_j - |f_j|^2 is computed with two bf16 matmul passes
    (hi*2hi, then hi*2lo + lo*2hi + ones*(-sq split into 3 bf16 rows)) that
    accumulate in fp32 PSUM -- bf16x3 product decomposition gives ~1e-5
    density accuracy, ~4x faster than native fp32 matmul.  The scalar engine
    evacuates PSUM->SBUF; the vector engine's MAX8 instruction returns the 8
    largest G per row (= 8 smallest d^2, row-shifted); the self element is
    always rank 0 and contributes exactly 0 to the mean, so density*8 =
    sum_{k=1..7} sqrt(relu(sq_i - v_k)), computed by two scalar-engine
    activations (Relu with per-partition bias, then Sqrt with accumulate).
Host does only O(B*N) work: input prep (transpose/bf16 split), the final
top-819 argsort of the 4096 densities per batch, and the output gather.
"""

from contextlib import ExitStack

import numpy as np

B, N, C = 8, 4096, 60
K, FACTOR = 8, 5
M = N // FACTOR
P = 128           # output row-tile partitions
NT = N // P       # row tiles per batch
JC = 512          # matmul moving-operand chunk (one PSUM bank)
KA = C            # pass-1 contraction rows
KB = 2 * C + 3    # pass-2 contraction rows

_CACHE = {}


def _build():
    import concourse.bass as bass
    import concourse.mybir as mybir
    import concourse.tile as tile
    from concourse import bacc

    F32 = mybir.dt.float32
    BF16 = mybir.dt.bfloat16
    nc = bacc.Bacc("TRN2", target_bir_lowering=False, debug=False)

    lhs1_d = nc.dram_tensor("lhs1", (KA, N), BF16, kind="ExternalInput").ap()
    lhs2_d = nc.dram_tensor("lhs2", (KB, N), BF16, kind="ExternalInput").ap()
    rhs1_d = nc.dram_tensor("rhs1", (KA, N), BF16, kind="ExternalInput").ap()
    rhs2_d = nc.dram_tensor("rhs2", (KB, N), BF16, kind="ExternalInput").ap()
    sqt_d = nc.dram_tensor("sqt", (P, NT), F32, kind="ExternalInput").ap()
    dens_d = nc.dram_tensor("dens", (P, NT), F32, kind="ExternalOutput").ap()

    QW = N // 4    # PSUM quarter width (quarter-granular release keeps
    NJQ = QW // JC  # the PE fed while the scalar engine evacuates)

    with tile.TileContext(nc) as tc, ExitStack() as ctx:
        const_pool = ctx.enter_context(tc.tile_pool(name="const", bufs=1))
        ps_pools = [ctx.enter_context(tc.tile_pool(
            name=f"ps{i}", bufs=1, space=bass.MemorySpace.PSUM))
            for i in range(4)]
        g_pool = ctx.enter_context(tc.tile_pool(name="g", bufs=3))
        s_pool = ctx.enter_context(tc.tile_pool(name="scr", bufs=4))

        lhs1 = const_pool.tile([KA, N], BF16)
        lhs2 = const_pool.tile([KB, N], BF16)
        rhs1 = const_pool.tile([KA, N], BF16)
        rhs2 = const_pool.tile([KB, N], BF16)
        sqt = const_pool.tile([P, NT], F32)
        dens = const_pool.tile([P, NT], F32)
        v_all = const_pool.tile([P, 8 * NT], F32)
        nc.sync.dma_start(lhs1[:], lhs1_d[:])
        nc.sync.dma_start(lhs2[:], lhs2_d[:])
        nc.sync.dma_start(rhs1[:], rhs1_d[:])
        nc.sync.dma_start(rhs2[:], rhs2_d[:])
        nc.sync.dma_start(sqt[:], sqt_d[:])

        for rt in range(NT):
            ms = bass.ts(rt, P)
            tiles = [pool.tile([P, QW], F32, name=f"pst{i}")
                     for i, pool in enumerate(ps_pools)]
            for pi, ps in enumerate(tiles):      # pass 1: hi . 2hi
                for j in range(NJQ):
                    nc.tensor.matmul(
                        ps[:, bass.ts(j, JC)], lhs1[:, ms],
                        rhs1[:, bass.ds(pi * QW + j * JC, JC)],
                        start=True, stop=False)
            for pi, ps in enumerate(tiles):      # pass 2: hi.2lo+lo.2hi-sq_j
                for j in range(NJQ):
                    nc.tensor.matmul(
                        ps[:, bass.ts(j, JC)], lhs2[:, ms],
                        rhs2[:, bass.ds(pi * QW + j * JC, JC)],
                        start=False, stop=True)
            g = g_pool.tile([P, N], F32)
            for pi, ps in enumerate(tiles):
                nc.scalar.copy(g[:, bass.ts(pi, QW)], ps[:])
            nc.vector.max(out=v_all[:, bass.ts(rt, 8)], in_=g[:])

        # deferred epilogue keeps DVE-dependent ops out of the ACT stream;
        # Relu clamps d2 = sq_i - v_k at 0 before Sqrt (guards the k-NN
        # d2 ~ 0 + rounding case; self is excluded via v[0])
        for rt in range(NT):
            d2t = s_pool.tile([P, 7], F32, name=f"d2t{rt}", tag="d2t")
            nc.scalar.activation(
                d2t[:], v_all[:, bass.ds(rt * 8 + 1, 7)],
                mybir.ActivationFunctionType.Relu,
                bias=sqt[:, rt:rt + 1], scale=-1.0)
            scr = s_pool.tile([P, 7], F32, name=f"scr{rt}", tag="scr")
            nc.scalar.activation(
                scr[:], d2t[:],
                mybir.ActivationFunctionType.Sqrt,
                accum_out=dens[:, rt:rt + 1])

        nc.sync.dma_start(dens_d[:], dens[:])

    nc.compile()
    return nc


def _prep_inputs(f):
    """f: (N, C) float32 -> device input dict (exact bf16 hi/lo split)."""
    import ml_dtypes
    bf16 = ml_dtypes.bfloat16
    f64 = f.astype(np.float64)
    hi = f.astype(bf16)
    lo = (f - hi.astype(np.float32)).astype(bf16)
    sq32 = np.einsum('nc,nc->n', f64, f64).astype(np.float32)
    s64 = sq32.astype(np.float64)
    sqh = sq32.astype(bf16)
    sqm = (s64 - sqh.astype(np.float64)).astype(np.float32).astype(bf16)
    sql = (s64 - sqh.astype(np.float64) - sqm.astype(np.float64)) \
        .astype(np.float32).astype(bf16)
    ones = np.ones((3, N), dtype=bf16)
    lhs1 = np.ascontiguousarray(hi.T)
    lhs2 = np.ascontiguousarray(np.concatenate([hi.T, lo.T, ones], axis=0))
    rhs1 = np.ascontiguousarray((hi * np.float32(2)).astype(bf16).T)
    lo2 = (lo * np.float32(2)).astype(bf16)
    hi2 = (hi * np.float32(2)).astype(bf16)
    rhs2 = np.ascontiguousarray(np.concatenate(
        [lo2.T, hi2.T, -sqh[None], -sqm[None], -sql[None]], axis=0))
    sqt = np.ascontiguousarray(sq32.reshape(NT, P).T)
    return {"lhs1": lhs1, "lhs2": lhs2, "rhs1": rhs1, "rhs2": rhs2,
            "sqt": sqt}


def kernel(features, pos, cam_ids):
    from concourse.bass_utils import run_bass_kernel_spmd

    features = np.asarray(features, dtype=np.float32)
    pos = np.asarray(pos, dtype=np.float32)
    cam_ids = np.asarray(cam_ids, dtype=np.int32)
    assert features.shape == (B, N, C), features.shape

    if "nc" not in _CACHE:
        _CACHE["nc"] = _build()
    nc = _CACHE["nc"]

    in_maps = [_prep_inputs(features[b]) for b in range(B)]
    res = run_bass_kernel_spmd(nc, in_maps, core_ids=list(range(B)))

    sf = np.empty((B, M, C), dtype=np.float32)
    sp = np.empty((B, M, 3), dtype=np.float32)
    sc = np.empty((B, M), dtype=np.int32)
    for b in range(B):
        dens = np.ascontiguousarray(res.results[b]["dens"].T).reshape(N)
        idx = np.argsort(-dens, kind="stable")[:M]
        sf[b] = features[b][idx]
        sp[b] = pos[b][idx]
        sc[b] = cam_ids[b][idx]
    return sf, sp, sc
